# revision 1
# baseline (speedup 1.0000x reference)
"""Dense transformer block (LN1 -> causal MHA -> residual -> LN2 -> MLP -> residual)
on 8 Trainium2 NeuronCores — single fused SPMD launch.

Sharding: core c <-> (batch b = c//2, head-half hh = c%2).
  Phase A: LN1 over all 2048 rows of batch b (affine folded into w_qkv on
           host), PE-transposed to lnxT [C, rows] bf16 in SBUF. (Replicated
           within the pair — cheap.)
  Phase B: qkv GEMM for the core's 8 heads (4 pair-groups of 2 heads stacked
           on partitions), all 2048 rows -> qT/kT/vT [128, 4, 2048] bf16.
  Phase C: PE-transpose vT to natural v layout, append a ones column per head
           (denominator trick).
  Phase D: flash-style causal attention in scoresT=[k,q] layout. Softmax
           max-subtraction skipped (scores bounded, |s|<~4 for this problem
           family). exp on ACT with the 1/sqrt(hd) scale folded in; causal
           masking by 0/1 mask multiply on diagonal tiles only; denominator
           comes out of the PV matmul via the ones column; normalization via
           reciprocal + PE broadcast matmul. -> ctxT [128, 4, 2048] bf16.
  Exchange: per-q-block pairwise AllGather (4 x 0.5MB bf16, fired as each
           q-block finishes so the wire time hides under attention); phase F
           selects its two q-blocks via partition-id-conditional DMAs.
  Phase F: out-proj + residual + LN2 (affine folded into w_fc) + fc + gelu +
           proj + residual, 256-row chunks.
All GEMM operands bf16, fp32 PSUM accumulation; stats in fp32.

Host keeps a cached jit executable; weights live device-side across calls.
"""

import hashlib

import numpy as np
import ml_dtypes

import jax
import jax.numpy as jnp
from jax.experimental.shard_map import shard_map
from jax.sharding import Mesh, NamedSharding, PartitionSpec as P

import concourse.bass as bass
import concourse.mybir as mybir
import concourse.tile as tile
from concourse import bacc, bass2jax

F32 = mybir.dt.float32
BF16 = mybir.dt.bfloat16
AF = mybir.ActivationFunctionType
BF16NP = ml_dtypes.bfloat16

C = 1024          # embed dim
NH = 16           # heads
HD = 64           # head dim
B, T = 4, 2048
R = B * T         # 8192 rows
NC = 8            # cores
RS = R // NC      # 1024 output rows per core
EPS = 1e-5
SCALE = 1.0 / np.sqrt(HD)
PAIRS = [[0, 1], [2, 3], [4, 5], [6, 7]]


def _ln_stats(nc, pool, x_tile, eps_sb):
    """bn_stats/bn_aggr over free dim (1024) -> per-partition rstd, -mu*rstd."""
    stats = pool.tile([128, 2, 6], F32, tag="bnstats")
    nc.vector.bn_stats(out=stats[:, 0, :], in_=x_tile[:, 0:512])
    nc.vector.bn_stats(out=stats[:, 1, :], in_=x_tile[:, 512:1024])
    mv = pool.tile([128, 2], F32, tag="bnaggr")
    nc.vector.bn_aggr(out=mv, in_=stats)
    rstd = pool.tile([128, 1], F32, tag="rstd")
    nc.scalar.activation(out=rstd, in_=mv[:, 1:2], func=AF.Sqrt, bias=eps_sb, scale=1.0)
    nc.vector.reciprocal(out=rstd, in_=rstd)
    nmr = pool.tile([128, 1], F32, tag="nmr")
    nc.vector.tensor_tensor(out=nmr, in0=mv[:, 0:1], in1=rstd, op=mybir.AluOpType.mult)
    nc.scalar.mul(out=nmr, in_=nmr, mul=-1.0)
    return rstd, nmr


def build_block(sim_act=False, repeat=0):
    nc = bacc.Bacc(num_devices=NC)
    x = nc.dram_tensor("x", [T, C], F32, kind="ExternalInput")        # batch rows
    xr = nc.dram_tensor("xr", [RS, C], F32, kind="ExternalInput")     # my F rows
    wqkv = nc.dram_tensor("wqkv", [C, 3 * 512], BF16, kind="ExternalInput")
    bqkv = nc.dram_tensor("bqkv", [128, 12], F32, kind="ExternalInput")
    masks = nc.dram_tensor("masks", [128, 4 * 1024], BF16, kind="ExternalInput")
    ident = nc.dram_tensor("ident", [128, 128], BF16, kind="ExternalInput")
    wo = nc.dram_tensor("wo", [C, C], BF16, kind="ExternalInput")
    bo = nc.dram_tensor("bo", [1, C], F32, kind="ExternalInput")
    wfc = nc.dram_tensor("wfc", [128, 8 * 4 * C], BF16, kind="ExternalInput")  # [p, ct, n] pre-arranged
    bfc = nc.dram_tensor("bfc", [128, 32], F32, kind="ExternalInput")
    wproj = nc.dram_tensor("wproj", [4 * C, C], BF16, kind="ExternalInput")
    bproj = nc.dram_tensor("bproj", [1, C], F32, kind="ExternalInput")
    out = nc.dram_tensor("out", [RS, C], F32, kind="ExternalOutput")

    xv = x.rearrange("(rt p) c -> p rt c", p=128)         # [128, 16, 1024]
    wv = wqkv.rearrange("(ct p) n -> p ct n", p=128)      # [128, 8, 1536]
    wov = wo.rearrange("(ct p) n -> p ct n", p=128)
    wfv = wfc.rearrange("p (ct n) -> p ct n", ct=8)    # [128, 8, 4096]
    wpv = wproj.rearrange("(ht p) n -> p ht n", p=128)
    xrv = xr.rearrange("(rt p) c -> p rt c", p=128)       # [128, 8, 1024]
    ov = out.rearrange("(rt p) c -> p rt c", p=128)

    with tile.TileContext(nc) as tc:
        with (
            tc.tile_pool(name="constp", bufs=1) as constp,
            tc.tile_pool(name="dram", bufs=1, space="DRAM") as dram,
        ):
            ident_sb = constp.tile([128, 128], BF16)
            nc.sync.dma_start(out=ident_sb, in_=ident[:, :])
            eps_sb = constp.tile([128, 1], F32)
            nc.vector.memset(eps_sb, EPS)
            ones_sb = constp.tile([1, 64], BF16)
            nc.vector.memset(ones_sb, 1.0)

            NQB = T // 512
            cc_in = [dram.tile([512, 512], BF16, name=f"cci{j}") for j in range(NQB)]
            cc_out = [dram.tile([1024, 512], BF16, name=f"cco{j}") for j in range(NQB)]

            # ================= phases A-D =================
            from contextlib import ExitStack
            loop_stack = ExitStack()
            if repeat:
                loop_stack.enter_context(
                    tc.For_i(0, repeat, 1, hint_engines=tuple(mybir.ALL_ENGINES)))
            ps_stack = ExitStack()
            with (
                tc.tile_pool(name="pa", bufs=1) as pa,
                tc.tile_pool(name="work", bufs=4) as work,
                tc.tile_pool(name="small", bufs=3) as small,
                tc.tile_pool(name="expp", bufs=4) as expp,
                tc.tile_pool(name="normp", bufs=6) as normp,
            ):
                ps = ps_stack.enter_context(tc.tile_pool(name="ps", bufs=2, space="PSUM"))
                pvps = ps_stack.enter_context(tc.tile_pool(name="pvps", bufs=3, space="PSUM"))
                bcps = ps_stack.enter_context(tc.tile_pool(name="bcps", bufs=1, space="PSUM"))

                # --- A-D merged flash pipeline over 512-row half-blocks:
                # LN1+transpose(hb) -> qkv(hb) -> v-transpose(hb) ->
                # attention q-block hb (its K/V prefix is fully available)
                # -> pair AllGather of that q-block's ctx rows. Each q-block's
                # ACT-heavy attention overlaps the next half-block's LN/qkv.
                lnxT_sb = pa.tile([128, 8, T], BF16)
                qkvT = [pa.tile([128, 4, T], BF16, name=f"qkv{s}") for s in range(3)]
                qT, kT, vT = qkvT
                vaug = pa.tile([128, 4, 16, 130], BF16)
                nc.vector.memset(vaug[:, :, :, 64:65], 1.0)
                nc.vector.memset(vaug[:, :, :, 129:130], 1.0)
                ctx_sb = pa.tile([128, 4, T], BF16)
                masks_sb = pa.tile([128, 4, 1024], BF16)
                wq_sb = pa.tile([128, 8, 3 * 512], BF16)
                bq_sb = pa.tile([128, 12], F32)
                for hb in range(4):
                    for rt in range(4 * hb, 4 * hb + 4):
                        x_sb = work.tile([128, C], F32, tag="x")
                        nc.sync.dma_start(out=x_sb, in_=xv[:, rt, :])
                        rstd, nmr = _ln_stats(nc, small, x_sb, eps_sb)
                        lnx = work.tile([128, C], BF16, tag="lnx")
                        nc.scalar.activation(out=lnx, in_=x_sb, func=AF.Identity,
                                             bias=nmr, scale=rstd)
                        for ct in range(8):
                            tp = ps.tile([128, 1024], BF16, tag="sc")
                            nc.tensor.transpose(tp[:, 0:128], lnx[:, ct * 128:(ct + 1) * 128],
                                                ident_sb)
                            nc.vector.tensor_copy(
                                out=lnxT_sb[:, ct, rt * 128:(rt + 1) * 128], in_=tp[:, 0:128])
                    if hb == 0:
                        # weight/mask loads deferred so the x tiles hit DMA first
                        nc.sync.dma_start(out=wq_sb, in_=wv[:, :, :])
                        nc.sync.dma_start(out=bq_sb, in_=bqkv[:, :])
                        nc.sync.dma_start(out=masks_sb,
                                          in_=masks.rearrange("p (d q) -> p d q", d=4))
                    for s in range(3):
                        for g in range(4):
                            mm = ps.tile([128, 1024], F32, tag="sc")
                            for ct in range(8):
                                nc.tensor.matmul(
                                    mm[:, 0:512],
                                    lhsT=wq_sb[:, ct, s * 512 + g * 128:s * 512 + (g + 1) * 128],
                                    rhs=lnxT_sb[:, ct, hb * 512:(hb + 1) * 512],
                                    start=(ct == 0), stop=(ct == 7),
                                )
                            nc.vector.tensor_scalar(
                                out=qkvT[s][:, g, hb * 512:(hb + 1) * 512], in0=mm[:, 0:512],
                                scalar1=bq_sb[:, s * 4 + g:s * 4 + g + 1], scalar2=None,
                                op0=mybir.AluOpType.add,
                            )
                    # v natural layout + ones columns for this block's k-tiles
                    for g in range(4):
                        for kt in range(4 * hb, 4 * hb + 4):
                            tp = ps.tile([128, 1024], BF16, tag="sc")
                            nc.tensor.transpose(tp[:, 0:128], vT[:, g, kt * 128:(kt + 1) * 128],
                                                ident_sb)
                            nc.vector.tensor_copy(out=vaug[:, g, kt, 0:64], in_=tp[:, 0:64])
                            nc.vector.tensor_copy(out=vaug[:, g, kt, 65:129], in_=tp[:, 64:128])
                    # attention for this q-block
                    for qb in (hb,):
                        for g in range(4):
                            pvA = pvps.tile([65, 512], F32, tag="pv", name="pvA")
                            pvB = pvps.tile([65, 512], F32, tag="pv", name="pvB")
                            nkt = 4 * qb + 4
                            for kt in range(nkt):
                                sc = ps.tile([128, 1024], F32, tag="sc")
                                for h in range(2):
                                    nc.tensor.matmul(
                                        sc[:, h * 512:(h + 1) * 512],
                                        lhsT=kT[h * 64:h * 64 + 64, g, kt * 128:(kt + 1) * 128],
                                        rhs=qT[h * 64:h * 64 + 64, g, qb * 512:(qb + 1) * 512],
                                        start=True, stop=True,
                                    )
                                et = expp.tile([128, 1024], BF16, tag="exp")
                                nc.scalar.activation(out=et, in_=sc, func=AF.Exp, scale=SCALE)
                                if kt >= 4 * qb:
                                    nc.vector.tensor_mul(et, et, masks_sb[:, kt - 4 * qb, :])
                                for h, pv in ((0, pvA), (1, pvB)):
                                    nc.tensor.matmul(
                                        pv,
                                        lhsT=vaug[:, g, kt, h * 65:(h + 1) * 65],
                                        rhs=et[:, h * 512:(h + 1) * 512],
                                        start=(kt == 0), stop=(kt == nkt - 1),
                                    )
                            for h, pv in ((0, pvA), (1, pvB)):
                                recip = normp.tile([1, 512], F32, tag="recip")
                                nc.vector.reciprocal(out=recip, in_=pv[64:65, :])
                                recb = normp.tile([1, 512], BF16, tag="recb")
                                nc.vector.tensor_copy(out=recb, in_=recip)
                                bc = bcps.tile([64, 512], F32, tag="bc")
                                nc.tensor.matmul(bc, lhsT=ones_sb, rhs=recb, start=True, stop=True)
                                bc_sb = normp.tile([64, 512], F32, tag="bcsb")
                                nc.vector.tensor_copy(out=bc_sb, in_=bc)
                                nc.vector.tensor_mul(
                                    ctx_sb[h * 64:(h + 1) * 64, g, qb * 512:(qb + 1) * 512],
                                    pv[0:64, :], bc_sb,
                                )
                        if qb == 3:
                            ps_stack.close()  # free PSUM banks for phase F
                        nc.sync.dma_start(
                            out=cc_in[qb][:, :].rearrange("(g p) r -> p g r", p=128),
                            in_=ctx_sb[:, :, qb * 512:(qb + 1) * 512],
                        )
                        if repeat:
                            # collectives can't sit inside a loop; substitute
                            # local DRAM copies for the timing build
                            nc.sync.dma_start(out=cc_out[qb][0:512, :], in_=cc_in[qb][:, :])
                            nc.sync.dma_start(out=cc_out[qb][512:1024, :], in_=cc_in[qb][:, :])
                        else:
                            nc.gpsimd.collective_compute(
                                "AllGather", mybir.AluOpType.bypass,
                                ins=[cc_in[qb][:, :]], outs=[cc_out[qb][:, :]],
                                replica_groups=PAIRS,
                            )

            # ================= phase F =================
            # cc_out[qb] holds full-C ctx for q-block qb; this core's rows are
            # q-blocks (2*(pid%2), 2*(pid%2)+1) — selected via conditional
            # DMAs (the program stays core-uniform).
            ccv = [co.rearrange("(blk g p) r -> p blk g r", p=128, g=4) for co in cc_out]
            pid = nc.partition_id()
            hhv = pid % 2              # 1 on odd cores
            nhv = (pid + 1) % 2        # 1 on even cores
            HRT = 2
            HROWS = HRT * 128
            with (
                tc.tile_pool(name="pf", bufs=1) as pf,
                tc.tile_pool(name="chunkp", bufs=2) as chunkp,
                tc.tile_pool(name="chunk1", bufs=1) as chunk1,
                tc.tile_pool(name="wstream", bufs=3) as wstream,
                tc.tile_pool(name="smallf", bufs=3) as smallf,
                tc.tile_pool(name="psf", bufs=3, space="PSUM") as psf,
                tc.tile_pool(name="psf2", bufs=2, space="PSUM") as psf2,
            ):
                # broadcast DMAs go on the ACT queue: gpsimd (Pool) would queue
                # them behind the collectives and stall phase F's start
                bo_sb = pf.tile([128, C], F32)
                nc.scalar.dma_start(out=bo_sb, in_=bo[0:1, :].to_broadcast([128, C]))
                bproj_sb = pf.tile([128, C], F32)
                nc.scalar.dma_start(out=bproj_sb, in_=bproj[0:1, :].to_broadcast([128, C]))
                bfc_sb = pf.tile([128, 32], F32)
                nc.sync.dma_start(out=bfc_sb, in_=bfc[:, :])
                wo_sb = pf.tile([128, 8, C], BF16)
                nc.sync.dma_start(out=wo_sb, in_=wov[:, :, :])
                wp_sb = pf.tile([128, 32, C], BF16)

                def emit_front(chunk):
                    """ctx DMA + out-proj + residual + LN2 + transpose."""
                    r0 = chunk * HRT
                    ctx_sb = chunkp.tile([128, 8, HROWS], BF16, tag="ctx", name="ctx_f")
                    qoff = (chunk * HROWS) % 512
                    for blk in range(2):
                        for j, cond in ((chunk // 2, nhv), (2 + chunk // 2, hhv)):
                            nc.sync.dma_start(
                                out=ctx_sb[:, blk * 4:(blk + 1) * 4, :],
                                in_=ccv[j][:, blk, :, qoff:qoff + HROWS],
                                cond=cond)
                    x_sb = chunkp.tile([128, HRT, C], F32, tag="x", name="x_f")
                    nc.sync.dma_start(out=x_sb, in_=xrv[:, r0:r0 + HRT, :])

                    xmid = chunkp.tile([128, HRT, C], F32, tag="xmid", name="xmid_f")
                    for rt in range(HRT):
                        for cb in range(2):
                            po = psf.tile([128, 512], F32, tag="mm")
                            for ct in range(8):
                                nc.tensor.matmul(
                                    po,
                                    lhsT=ctx_sb[:, ct, rt * 128:(rt + 1) * 128],
                                    rhs=wo_sb[:, ct, cb * 512:(cb + 1) * 512],
                                    start=(ct == 0), stop=(ct == 7),
                                )
                            sl = slice(cb * 512, (cb + 1) * 512)
                            nc.vector.tensor_add(out=po, in0=po, in1=bo_sb[:, sl])
                            nc.vector.tensor_add(out=xmid[:, rt, sl], in0=po, in1=x_sb[:, rt, sl])

                    ln2T = chunkp.tile([128, 8, HROWS], BF16, tag="ln2T", name="ln2T_f")
                    for rt in range(HRT):
                        rstd, nmr = _ln_stats(nc, smallf, xmid[:, rt, :], eps_sb)
                        lnx = smallf.tile([128, C], BF16, tag="lnx")
                        nc.scalar.activation(out=lnx, in_=xmid[:, rt, :], func=AF.Identity,
                                             bias=nmr, scale=rstd)
                        for ct in range(8):
                            tp = psf.tile([128, 128], BF16, tag="mm")
                            nc.tensor.transpose(tp, lnx[:, ct * 128:(ct + 1) * 128], ident_sb)
                            nc.vector.tensor_copy(out=ln2T[:, ct, rt * 128:(rt + 1) * 128], in_=tp)
                    return ln2T, xmid

                def emit_back(chunk, ln2T, xmid):
                    """fc + gelu + proj + residual + out DMA."""
                    r0 = chunk * HRT
                    hT = chunk1.tile([128, 32, HROWS], BF16, tag="hT", name="hT_f")
                    for htg in range(8):   # 4 fc-tiles per weight DMA
                        wf_sb = wstream.tile([128, 8, 512], BF16, tag="wfc", name="wf_sb")
                        nc.sync.dma_start(out=wf_sb, in_=wfv[:, :, htg * 512:(htg + 1) * 512])
                        for hi in range(4):
                            ht = htg * 4 + hi
                            pfc = psf2.tile([128, HROWS], F32, tag="fc")
                            for ct in range(8):
                                nc.tensor.matmul(
                                    pfc,
                                    lhsT=wf_sb[:, ct, hi * 128:(hi + 1) * 128],
                                    rhs=ln2T[:, ct, :],
                                    start=(ct == 0), stop=(ct == 7),
                                )
                            nc.scalar.activation(out=hT[:, ht, :], in_=pfc,
                                                 func=AF.Identity if sim_act else AF.Gelu,
                                                 bias=bfc_sb[:, ht:ht + 1], scale=1.0)

                    o_sb = chunk1.tile([128, HRT, C], F32, tag="o", name="o_f")
                    for rt in range(HRT):
                        for cb in range(2):
                            pp = psf.tile([128, 512], F32, tag="mm")
                            for ht in range(32):
                                nc.tensor.matmul(
                                    pp,
                                    lhsT=hT[:, ht, rt * 128:(rt + 1) * 128],
                                    rhs=wp_sb[:, ht, cb * 512:(cb + 1) * 512],
                                    start=(ht == 0), stop=(ht == 31),
                                )
                            sl = slice(cb * 512, (cb + 1) * 512)
                            nc.vector.tensor_add(out=pp, in0=pp, in1=bproj_sb[:, sl])
                            nc.vector.tensor_add(out=o_sb[:, rt, sl], in0=pp, in1=xmid[:, rt, sl])
                        nc.sync.dma_start(out=ov[:, r0 + rt, :], in_=o_sb[:, rt, :])

                nchunks = RS // HROWS
                f0 = (0, *emit_front(0))
                f1 = (1, *emit_front(1))
                # wproj load issued after the first ctx DMAs so it doesn't
                # gate phase F's start; it completes under fc(chunk 0).
                nc.sync.dma_start(out=wp_sb, in_=wpv[:, :, :])
                emit_back(*f0)
                f2 = (2, *emit_front(2))
                emit_back(*f1)
                f3 = (3, *emit_front(3))
                emit_back(*f2)
                emit_back(*f3)
            loop_stack.close()
    nc.compile()
    return nc


class Runner:
    """Cached-jit SPMD runner (replaces run_bass_kernel_spmd for repeat calls).

    Mimics bass2jax.run_bass_via_pjrt but jits once and keeps constant inputs
    device-resident across calls.
    """

    def __init__(self, nc, n_cores=NC):
        bass2jax.install_neuronx_cc_hook()
        self.nc = nc
        self.n_cores = n_cores
        in_names, out_names, out_avals, zero_info = [], [], [], []
        partition_name = nc.partition_id_tensor.name if nc.partition_id_tensor else None
        for alloc in nc.m.functions[0].allocations:
            if not isinstance(alloc, mybir.MemoryLocationSet):
                continue
            name = alloc.memorylocations[0].name
            if alloc.kind == "ExternalInput":
                if name != partition_name:
                    in_names.append(name)
            elif alloc.kind == "ExternalOutput":
                out_names.append(name)
                shape = tuple(alloc.tensor_shape)
                dtype = mybir.dt.np(alloc.dtype)
                out_avals.append(jax.core.ShapedArray(shape, dtype))
                zero_info.append((shape, dtype))
        self.in_names = list(in_names)
        self.out_names = out_names
        n_params = len(in_names)
        n_outs = len(out_names)
        all_in_names = in_names + out_names
        if partition_name is not None:
            all_in_names.append(partition_name)

        devices = jax.devices()[:n_cores]
        self.mesh = Mesh(np.asarray(devices), ("core",))
        self.sharding = NamedSharding(self.mesh, P("core"))

        def _body(*args):
            operands = list(args)
            if partition_name is not None:
                operands.append(bass2jax.partition_id_tensor())
            outs = bass2jax._bass_exec_p.bind(
                *operands,
                out_avals=tuple(out_avals),
                in_names=tuple(all_in_names),
                out_names=tuple(out_names),
                lowering_input_output_aliases=(),
                sim_require_finite=False,
                sim_require_nnan=False,
                nc=nc,
            )
            return tuple(outs)

        in_specs = (P("core"),) * (n_params + n_outs)
        out_specs = (P("core"),) * n_outs
        donate = tuple(range(n_params, n_params + n_outs))
        self.fn = jax.jit(
            shard_map(_body, mesh=self.mesh, in_specs=in_specs,
                      out_specs=out_specs, check_rep=False),
            donate_argnums=donate, keep_unused=True,
        )
        shardings = tuple(self.sharding for _ in zero_info)
        self.zeros_fn = jax.jit(
            lambda: tuple(jnp.zeros((n_cores * s[0], *s[1:]), d) for s, d in zero_info),
            out_shardings=shardings if zero_info else None,
        )
        self._dev_cache = {}

    def put(self, name, per_core_arrays):
        """Device-put a (replicated-or-not) input once; cached by name."""
        glob = np.concatenate([np.asarray(a) for a in per_core_arrays], axis=0)
        self._dev_cache[name] = jax.device_put(glob, self.sharding)

    def __call__(self, var_inputs):
        """var_inputs: dict name -> list of per-core np arrays (for inputs not
        previously .put()). Returns list of per-core dicts of np outputs."""
        args = []
        for name in self.in_names:
            if name in var_inputs:
                glob = np.concatenate([np.asarray(a) for a in var_inputs[name]], axis=0)
                args.append(glob)
            else:
                args.append(self._dev_cache[name])
        zeros = self.zeros_fn()
        outs = self.fn(*args, *zeros)
        res = []
        for c in range(self.n_cores):
            d = {}
            for i, name in enumerate(self.out_names):
                arr = np.asarray(outs[i])
                per = arr.shape[0] // self.n_cores
                d[name] = arr[c * per:(c + 1) * per]
            res.append(d)
        return res


_CACHE = {}


def _consts():
    if "consts" not in _CACHE:
        ident = np.eye(128, dtype=BF16NP)
        kk = np.arange(128)[:, None]
        qq = np.arange(512)[None, :]
        # per delta: [128, 1024] = the same [128, 512] mask duplicated for the
        # two heads packed side by side in the paired score tile
        masks = np.concatenate(
            [np.tile((qq >= kk + d).astype(BF16NP), (1, 2)) for d in (0, 128, 256, 384)],
            axis=1)
        _CACHE["consts"] = (ident, masks)
    return _CACHE["consts"]


def kernel(x, ln1_w, ln1_b, w_qkv, b_qkv, w_o, b_o,
           ln2_w, ln2_b, w_fc, b_fc, w_proj, b_proj):
    x = np.asarray(x, np.float32)
    x2 = np.ascontiguousarray(x.reshape(R, C))
    ident, masks = _consts()

    h = hashlib.blake2b(digest_size=8)
    for a in (ln1_w, ln1_b, w_qkv, b_qkv, w_o, b_o, ln2_w, ln2_b, w_fc, b_fc,
              w_proj, b_proj):
        h.update(np.ascontiguousarray(np.asarray(a, np.float32)).data)
    wkey = h.hexdigest()

    if _CACHE.get("runner") is None:
        nc = build_block()
        _CACHE["runner"] = Runner(nc)
    runner = _CACHE["runner"]

    if _CACHE.get("wkey") != wkey:
        w_qkv = np.asarray(w_qkv, np.float32)
        w_fc = np.asarray(w_fc, np.float32)
        wqkv_eff = w_qkv * np.asarray(ln1_w, np.float32)[:, None]
        bqkv_eff = np.asarray(b_qkv, np.float32) + np.asarray(ln1_b, np.float32) @ w_qkv
        wfc_eff = w_fc * np.asarray(ln2_w, np.float32)[:, None]
        bfc_eff = np.asarray(b_fc, np.float32) + np.asarray(ln2_b, np.float32) @ w_fc

        wq_cores, bq_cores = [], []
        for c in range(NC):
            hh = c % 2
            sl = slice(hh * 512, (hh + 1) * 512)
            wq_cores.append(np.ascontiguousarray(np.concatenate(
                [wqkv_eff[:, 0 * C:1 * C][:, sl], wqkv_eff[:, 1 * C:2 * C][:, sl],
                 wqkv_eff[:, 2 * C:3 * C][:, sl]], axis=1).astype(BF16NP)))
            bq = np.zeros((128, 12), np.float32)
            for s in range(3):
                for g in range(4):
                    bq[:, s * 4 + g] = bqkv_eff[s * C + hh * 512 + g * 128:
                                                s * C + hh * 512 + (g + 1) * 128]
            bq_cores.append(bq)

        runner.put("wqkv", wq_cores)
        runner.put("bqkv", bq_cores)
        runner.put("masks", [masks] * NC)
        runner.put("ident", [ident] * NC)
        runner.put("wo", [np.asarray(w_o, np.float32).astype(BF16NP)] * NC)
        runner.put("bo", [np.asarray(b_o, np.float32).reshape(1, C)] * NC)
        wfc_arr = np.ascontiguousarray(wfc_eff.astype(BF16NP).reshape(8, 128, 4 * C)
                                       .transpose(1, 0, 2).reshape(128, 8 * 4 * C))
        runner.put("wfc", [wfc_arr] * NC)
        runner.put("bfc", [np.ascontiguousarray(bfc_eff.reshape(32, 128).T)] * NC)
        runner.put("wproj", [np.asarray(w_proj, np.float32).astype(BF16NP)] * NC)
        runner.put("bproj", [np.asarray(b_proj, np.float32).reshape(1, C)] * NC)
        _CACHE["wkey"] = wkey

    xkey = hashlib.blake2b(x2.data, digest_size=8).hexdigest()
    if _CACHE.get("xkey") != xkey:
        runner.put("x", [x2[(c // 2) * T:(c // 2 + 1) * T] for c in range(NC)])
        runner.put("xr", [x2[c * RS:(c + 1) * RS] for c in range(NC)])
        _CACHE["xkey"] = xkey

    res = runner({})
    out = np.concatenate([res[c]["out"] for c in range(NC)], axis=0)
    return out.reshape(B, T, C)



# revision 9
# speedup vs baseline: 1.5446x; 1.5446x over previous
"""Dense transformer block (LN1 -> causal MHA -> residual -> LN2 -> MLP -> residual)
on 8 Trainium2 NeuronCores — single fused SPMD launch.

Sharding: core c <-> (batch b = c//2, head-half hh = c%2).
  Phase A: LN1 over all 2048 rows of batch b (affine folded into w_qkv on
           host), PE-transposed to lnxT [C, rows] fp8-e4m3 in SBUF.
  Phase B: qkv GEMM for the core's 8 heads via fp8 DoubleRow matmuls
           (K=256 per pass, weights pre-scaled x16 into e4m3; the x16 on
           q/k folds into the exp scale, v is descaled during the vaug
           copy). All 2048 rows -> qT/kT [128, 4, 2048] bf16 (16x true),
           vT bf16.
  Phase C: PE-transpose vT to natural v layout in fp8 (x 1/16 descale on
           the Pool engine), append a ones column per head (denominator
           trick).
  Phase D: flash-style causal attention over 512-row q-blocks:
           scoresT=[k,q] bf16 matmuls (K=64, N=512; tiled-PE matmuls at
           N=256 fault the device); exp on ACT with scale/256 and a -ln4
           bias folded in, written as fp8 (via bf16 + mask-multiply on
           the 4 diagonal k-tiles); PV via fp8 DoubleRow over kt-pairs;
           denominator from the ones column; normalization via
           reciprocal + PE broadcast matmul, ctx written fp8.
  Exchange: 4 relative-chunk pairwise AllGathers in fp8. AG_k carries this
           core's 8 heads' ctx for 256-row chunk k of BOTH halves of the
           pair's rows, so phase-F chunk k depends on exactly AG_k on both
           cores (no cross-AG false deps). AG0/AG1 fire after attention
           q-block 2, AG2/AG3 after q-block 3.
  Phase F: out-proj (fp8 DoubleRow over ct-pairs, x 1/16 descale in the
           bias stage) + residual + LN2 (affine folded into w_fc) + fc +
           gelu + proj + residual, 256-row chunks; fc/proj stay bf16
           (fp8 there costs ~1.7e-2 rel err - measured, too risky).
All other GEMM operands bf16, fp32 PSUM accumulation; stats in fp32.

fp8 notes (validated on HW): DoubleRow needs both operands e4m3/e5m2 and
gives ~2x effective PE throughput (same ~234ns per 512-wide matmul as
bf16, at double contraction depth). e4m3 here is ml_dtypes.float8_e4m3
(IEEE, max 240): max |exp(s/8 - ln4)| measured 2.9, weights x16 max ~1.6.

Host keeps a cached jit executable; weights live device-side across calls.
Each Runner pins a fresh NEURON_COMPILE_CACHE_URL: the neuron jit cache
keys on the outer HLO only (NOT the embedded bass program), so two builds
with identical I/O signatures would otherwise silently reuse a stale NEFF.
"""

import hashlib
import os
import tempfile

import numpy as np
import ml_dtypes

import jax
import jax.numpy as jnp
from jax.experimental.shard_map import shard_map
from jax.sharding import Mesh, NamedSharding, PartitionSpec as P

import concourse.bass as bass
import concourse.mybir as mybir
import concourse.tile as tile
from concourse import bacc, bass2jax

F32 = mybir.dt.float32
BF16 = mybir.dt.bfloat16
E4 = mybir.dt.float8e4
AF = mybir.ActivationFunctionType
DR = mybir.MatmulPerfMode.DoubleRow
BF16NP = ml_dtypes.bfloat16
E4NP = ml_dtypes.float8_e4m3

C = 1024          # embed dim
NH = 16           # heads
HD = 64           # head dim
B, T = 4, 2048
R = B * T         # 8192 rows
NC = 8            # cores
RS = R // NC      # 1024 output rows per core
EPS = 1e-5
WS = 16.0         # fp8 weight pre-scale
SCALE = 1.0 / np.sqrt(HD)
EXP_BIAS = -float(np.log(4.0))   # exp headroom: keeps et <= ~3 << 240
PAIRS = [[0, 1], [2, 3], [4, 5], [6, 7]]


def _ln_stats(nc, pool, x_tile, eps_sb):
    """bn_stats/bn_aggr over free dim (1024) -> per-partition rstd, -mu*rstd."""
    stats = pool.tile([128, 2, 6], F32, tag="bnstats")
    nc.vector.bn_stats(out=stats[:, 0, :], in_=x_tile[:, 0:512])
    nc.vector.bn_stats(out=stats[:, 1, :], in_=x_tile[:, 512:1024])
    mv = pool.tile([128, 2], F32, tag="bnaggr")
    nc.vector.bn_aggr(out=mv, in_=stats)
    rstd = pool.tile([128, 1], F32, tag="rstd")
    nc.scalar.activation(out=rstd, in_=mv[:, 1:2], func=AF.Sqrt, bias=eps_sb, scale=1.0)
    nc.vector.reciprocal(out=rstd, in_=rstd)
    nmr = pool.tile([128, 1], F32, tag="nmr")
    nc.vector.tensor_tensor(out=nmr, in0=mv[:, 0:1], in1=rstd, op=mybir.AluOpType.mult)
    nc.scalar.mul(out=nmr, in_=nmr, mul=-1.0)
    return rstd, nmr


def build_block(sim_act=False, repeat=0, parts='all'):
    nc = bacc.Bacc(num_devices=NC)
    x = nc.dram_tensor("x", [T, C], F32, kind="ExternalInput")        # batch rows
    xr = nc.dram_tensor("xr", [RS, C], F32, kind="ExternalInput")     # my F rows
    wqkv = nc.dram_tensor("wqkv", [C, 3 * 512], E4, kind="ExternalInput")
    bqkv = nc.dram_tensor("bqkv", [128, 12], F32, kind="ExternalInput")
    masks = nc.dram_tensor("masks", [128, 4 * 1024], BF16, kind="ExternalInput")
    ident = nc.dram_tensor("ident", [128, 128], BF16, kind="ExternalInput")
    wo = nc.dram_tensor("wo", [C, C], E4, kind="ExternalInput")
    bo = nc.dram_tensor("bo", [1, C], F32, kind="ExternalInput")
    wfc = nc.dram_tensor("wfc", [128, 8 * 4 * C], BF16, kind="ExternalInput")  # [p, ct, n]
    bfc = nc.dram_tensor("bfc", [128, 32], F32, kind="ExternalInput")
    wproj = nc.dram_tensor("wproj", [4 * C, C], BF16, kind="ExternalInput")
    bproj = nc.dram_tensor("bproj", [1, C], F32, kind="ExternalInput")
    out = nc.dram_tensor("out", [RS, C], F32, kind="ExternalOutput")

    xv = x.rearrange("(rt p) c -> p rt c", p=128)         # [128, 16, 1024]
    wv = wqkv.rearrange("(ct p) n -> p ct n", p=128)      # [128, 8, 1536]
    wov = wo.rearrange("(ct p) n -> p ct n", p=128)
    wfv = wfc.rearrange("p (ct n) -> p ct n", ct=8)       # [128, 8, 4096]
    wpv = wproj.rearrange("(ht p) n -> p ht n", p=128)
    xrv = xr.rearrange("(rt p) c -> p rt c", p=128)       # [128, 8, 1024]
    ov = out.rearrange("(rt p) c -> p rt c", p=128)

    with tile.TileContext(nc) as tc:
        with (
            tc.tile_pool(name="constp", bufs=1) as constp,
            tc.tile_pool(name="dram", bufs=1, space="DRAM") as dram,
        ):
            ident_sb = constp.tile([128, 128], BF16)
            nc.sync.dma_start(out=ident_sb, in_=ident[:, :])
            eps_sb = constp.tile([128, 1], F32)
            nc.vector.memset(eps_sb, EPS)
            ones_sb = constp.tile([1, 64], BF16)
            nc.vector.memset(ones_sb, 1.0)
            expb_sb = constp.tile([128, 1], F32)
            nc.vector.memset(expb_sb, EXP_BIAS)

            # relative-chunk exchange buffers: AG_k = my heads' ctx for
            # 256-row chunk k of the even half (rows 0-511) and odd half
            # (rows 512-1023); gathered out has the pair's two cores stacked.
            cc_in = [dram.tile([1024, 256], E4, name=f"cci{j}") for j in range(4)]
            cc_out = [dram.tile([2048, 256], E4, name=f"cco{j}") for j in range(4)]

            # ================= phases A-D =================
            from contextlib import ExitStack
            loop_stack = ExitStack()
            if repeat:
                loop_stack.enter_context(
                    tc.For_i(0, repeat, 1, hint_engines=tuple(mybir.ALL_ENGINES)))
            ps_stack = ExitStack()
            with (
                tc.tile_pool(name="pa", bufs=1) as pa,
                tc.tile_pool(name="work", bufs=6) as work,
                tc.tile_pool(name="small", bufs=6) as small,
                tc.tile_pool(name="expp", bufs=4) as expp,
                tc.tile_pool(name="etbp", bufs=2) as etbp,
                tc.tile_pool(name="normp", bufs=6) as normp,
            ):
                ps = ps_stack.enter_context(tc.tile_pool(name="ps", bufs=2, space="PSUM"))
                pvps = ps_stack.enter_context(tc.tile_pool(name="pvps", bufs=3, space="PSUM"))
                bcps = ps_stack.enter_context(tc.tile_pool(name="bcps", bufs=1, space="PSUM"))

                lnxT_sb = pa.tile([128, 8, T], E4)
                qT = pa.tile([128, 4, T], BF16, name="qT")
                kT = pa.tile([128, 4, T], BF16, name="kT")
                vT = pa.tile([128, 4, T], BF16, name="vT")
                # [g, kt, h, 72]: per-kt stride 144 (16B-aligned for dual-fp8
                # ldweights), per-head 64 v cols + ones col at 64 + 7 pad
                vaug = pa.tile([128, 4, 16, 2, 72], E4)
                nc.vector.memset(vaug[:, :, :, :, 64:65], 1.0)
                ctx_sb = pa.tile([128, 4, T], E4)
                masks_sb = pa.tile([128, 4, 1024], BF16)
                wq_sb = pa.tile([128, 8, 3 * 512], E4)
                bq_sb = pa.tile([128, 12], F32)

                def emit_attn(qb, skip_pv=False):
                    """Causal attention for q-block qb (512 rows); scoresT
                    bf16 (K=64, N=512 - tiled-PE matmuls at N=256 fault the
                    device, so q stays 512 wide); exp -> fp8 on ACT (bf16 +
                    mask multiply on the 4 diagonal k-tiles); PV via fp8
                    DoubleRow over kt-pairs."""
                    for g in range(4):
                        pvA = pvps.tile([65, 512], F32, tag="pv", name="pvA")
                        pvB = pvps.tile([65, 512], F32, tag="pv", name="pvB")
                        nktp = 2 * qb + 2
                        for ktp in range(nktp):
                            et = expp.tile([128, 2, 2, 512], E4, tag="et")  # [t, h, q]
                            for t in range(2):
                                kt = ktp * 2 + t
                                sc = ps.tile([128, 1024], F32, tag="sc")
                                for h in range(2):
                                    nc.tensor.matmul(
                                        sc[:, h * 512:(h + 1) * 512],
                                        lhsT=kT[h * 64:h * 64 + 64, g, kt * 128:(kt + 1) * 128],
                                        rhs=qT[h * 64:h * 64 + 64, g, qb * 512:(qb + 1) * 512],
                                        start=True, stop=True,
                                    )
                                if kt >= 4 * qb:
                                    etb = etbp.tile([128, 1024], BF16, tag="etb")
                                    nc.scalar.activation(out=etb, in_=sc, func=AF.Exp,
                                                         scale=SCALE / (WS * WS), bias=expb_sb)
                                    meng = nc.gpsimd if (g + kt) % 2 else nc.vector
                                    meng.tensor_mul(
                                        et[:, t, :, :].rearrange("p b q -> p (b q)"),
                                        etb, masks_sb[:, kt - 4 * qb, :])
                                else:
                                    nc.scalar.activation(
                                        out=et[:, t, :, :].rearrange("p b q -> p (b q)"),
                                        in_=sc, func=AF.Exp,
                                        scale=SCALE / (WS * WS), bias=expb_sb)
                            if skip_pv:
                                continue
                            for h, pv in ((0, pvA), (1, pvB)):
                                nc.tensor.matmul(
                                    pv,
                                    lhsT=vaug[:, g, ktp * 2:ktp * 2 + 2, h, 0:65],
                                    rhs=et[:, :, h, :],
                                    start=(ktp == 0), stop=(ktp == nktp - 1),
                                    perf_mode=DR,
                                )
                        if skip_pv:
                            continue
                        for h, pv in ((0, pvA), (1, pvB)):
                            recip = normp.tile([1, 512], F32, tag="recip")
                            nc.vector.reciprocal(out=recip, in_=pv[64:65, :])
                            recb = normp.tile([1, 512], BF16, tag="recb")
                            nc.vector.tensor_copy(out=recb, in_=recip)
                            bc = bcps.tile([64, 512], F32, tag="bc")
                            nc.tensor.matmul(bc, lhsT=ones_sb, rhs=recb, start=True, stop=True)
                            bc_sb = normp.tile([64, 512], F32, tag="bcsb")
                            nc.vector.tensor_copy(out=bc_sb, in_=bc)
                            nc.vector.tensor_mul(
                                ctx_sb[h * 64:(h + 1) * 64, g, qb * 512:(qb + 1) * 512],
                                pv[0:64, :], bc_sb,
                            )

                def emit_ag(k):
                    """cc_in[k] rows 0-511: my heads, even-half chunk k;
                    rows 512-1023: odd-half chunk k. AllGather with the pair."""
                    for half in range(2):
                        q0 = (2 * half + k // 2) * 512 + (k % 2) * 256
                        nc.sync.dma_start(
                            out=cc_in[k][half * 512:(half + 1) * 512, :]
                                .rearrange("(g p) r -> p g r", p=128),
                            in_=ctx_sb[:, :, q0:q0 + 256],
                        )
                    if repeat:
                        # collectives can't sit inside a loop; substitute
                        # equivalent local DRAM copies for the timing build
                        nc.sync.dma_start(out=cc_out[k][0:1024, :], in_=cc_in[k][:, :])
                        nc.sync.dma_start(out=cc_out[k][1024:2048, :], in_=cc_in[k][:, :])
                    else:
                        nc.gpsimd.collective_compute(
                            "AllGather", mybir.AluOpType.bypass,
                            ins=[cc_in[k][:, :]], outs=[cc_out[k][:, :]],
                            replica_groups=PAIRS,
                        )

                for hb in range(4):
                    # LN1 + transpose for this half-block's 4 row-tiles
                    for rt in range(4 * hb, 4 * hb + 4):
                        x_sb = work.tile([128, C], F32, tag="x")
                        nc.sync.dma_start(out=x_sb, in_=xv[:, rt, :])
                        rstd, nmr = _ln_stats(nc, small, x_sb, eps_sb)
                        lnx = work.tile([128, C], BF16, tag="lnx")
                        nc.scalar.activation(out=lnx, in_=x_sb, func=AF.Identity,
                                             bias=nmr, scale=rstd)
                        tp8 = ps.tile([128, 8, 128], BF16, tag="sc")
                        for ct in range(8):
                            nc.tensor.transpose(tp8[:, ct, :], lnx[:, ct * 128:(ct + 1) * 128],
                                                ident_sb)
                        # one strided ACT copy per row-tile: PSUM bf16 -> fp8
                        # (gpsimd cannot read PSUM on TRN2)
                        nc.scalar.activation(out=lnxT_sb[:, :, rt * 128:(rt + 1) * 128],
                                             in_=tp8, func=AF.Identity, bias=0.0, scale=1.0)
                    if hb == 0:
                        # weight/mask loads deferred so the x tiles hit DMA first
                        nc.sync.dma_start(out=wq_sb, in_=wv[:, :, :])
                        nc.sync.dma_start(out=bq_sb, in_=bqkv[:, :])
                        nc.sync.dma_start(out=masks_sb,
                                          in_=masks.rearrange("p (d q) -> p d q", d=4))
                    # qkv for this half-block: fp8 DoubleRow, K=256 per pass
                    for s, dst in ((0, qT), (1, kT), (2, vT)):
                        for g in range(4):
                            mm = ps.tile([128, 1024], F32, tag="sc")
                            for c2 in range(4):
                                nc.tensor.matmul(
                                    mm[:, 0:512],
                                    lhsT=wq_sb[:, 2 * c2:2 * c2 + 2,
                                               s * 512 + g * 128:s * 512 + (g + 1) * 128],
                                    rhs=lnxT_sb[:, 2 * c2:2 * c2 + 2, hb * 512:(hb + 1) * 512],
                                    start=(c2 == 0), stop=(c2 == 3),
                                    perf_mode=DR,
                                )
                            nc.vector.tensor_scalar(
                                out=dst[:, g, hb * 512:(hb + 1) * 512], in0=mm[:, 0:512],
                                scalar1=bq_sb[:, s * 4 + g:s * 4 + g + 1], scalar2=None,
                                op0=mybir.AluOpType.add,
                            )
                    # v natural layout (descaled x1/16 to true scale, fp8)
                    # + ones columns, for this block's k-tiles
                    for g in range(4):
                        tp4 = ps.tile([128, 4, 128], BF16, tag="sc")
                        for j in range(4):
                            kt = 4 * hb + j
                            nc.tensor.transpose(tp4[:, j, :], vT[:, g, kt * 128:(kt + 1) * 128],
                                                ident_sb)
                        nc.vector.tensor_scalar_mul(
                            out=vaug[:, g, 4 * hb:4 * hb + 4, 0, 0:64],
                            in0=tp4[:, :, 0:64], scalar1=1.0 / WS)
                        nc.vector.tensor_scalar_mul(
                            out=vaug[:, g, 4 * hb:4 * hb + 4, 1, 0:64],
                            in0=tp4[:, :, 64:128], scalar1=1.0 / WS)
                    # attention + exchanges per schedule
                    if parts in ('all', 'noF', 'attnSE'):
                        emit_attn(hb, skip_pv=(parts == 'attnSE'))
                    elif hb == 0:
                        nc.vector.memset(ctx_sb, 0.0)
                    if parts == 'attnSE' and hb == 0:
                        nc.vector.memset(ctx_sb, 0.0)
                    if hb == 2:
                        emit_ag(0)
                        emit_ag(1)
                    if hb == 3:
                        ps_stack.close()  # free PSUM banks for phase F
                        emit_ag(2)
                        emit_ag(3)

            if parts in ('AB', 'noF', 'attnSE'):
                nc.sync.dma_start(out=out[:, :], in_=xr[:, :])
            # ================= phase F =================
            # cc_out[k] holds the pair's full-C ctx for 256-row chunk k of
            # both halves; this core's rows select via partition-id
            # conditional DMAs (the program stays core-uniform, and both
            # cond variants wait on the same AG_k).
            ccv = [co.rearrange("(blk g p) r -> p blk g r", p=128, g=4) for co in cc_out]
            if parts in ('AB', 'noF'):
                ccv = ccv  # phase F skipped; out already written from xr
            pid = nc.partition_id()
            hhv = pid % 2              # 1 on odd cores
            nhv = (pid + 1) % 2        # 1 on even cores
            HRT = 2
            HROWS = HRT * 128
            run_F = parts in ('all',)
            with (
                tc.tile_pool(name="pf", bufs=1) as pf,
                tc.tile_pool(name="chunkp", bufs=2) as chunkp,
                tc.tile_pool(name="chunk1", bufs=1) as chunk1,
                tc.tile_pool(name="wstream", bufs=3) as wstream,
                tc.tile_pool(name="smallf", bufs=3) as smallf,
                tc.tile_pool(name="psf", bufs=3, space="PSUM") as psf,
                tc.tile_pool(name="psf2", bufs=2, space="PSUM") as psf2,
            ):
                # broadcast DMAs go on the ACT queue: gpsimd (Pool) would
                # queue them behind the collectives and stall phase F's start
                bo_sb = pf.tile([128, C], F32)
                nc.scalar.dma_start(out=bo_sb, in_=bo[0:1, :].to_broadcast([128, C]))
                bproj_sb = pf.tile([128, C], F32)
                nc.scalar.dma_start(out=bproj_sb, in_=bproj[0:1, :].to_broadcast([128, C]))
                bfc_sb = pf.tile([128, 32], F32)
                nc.sync.dma_start(out=bfc_sb, in_=bfc[:, :])
                wo_sb = pf.tile([128, 8, C], E4)
                nc.sync.dma_start(out=wo_sb, in_=wov[:, :, :])
                wp_sb = pf.tile([128, 32, C], BF16)

                def emit_front(chunk):
                    """ctx DMA + out-proj (fp8 DR) + residual + LN2 + transpose."""
                    r0 = chunk * HRT
                    ctx_f = chunkp.tile([128, 8, HROWS], E4, tag="ctx", name="ctx_f")
                    for cthalf, blk0 in ((0, 0), (1, 2)):
                        for blk, cond in ((blk0, nhv), (blk0 + 1, hhv)):
                            nc.sync.dma_start(
                                out=ctx_f[:, cthalf * 4:(cthalf + 1) * 4, :],
                                in_=ccv[chunk][:, blk, :, :],
                                cond=cond)
                    x_sb = chunkp.tile([128, HRT, C], F32, tag="x", name="x_f")
                    nc.sync.dma_start(out=x_sb, in_=xrv[:, r0:r0 + HRT, :])
                    xbo = chunkp.tile([128, HRT, C], F32, tag="xbo", name="xbo_f")
                    for rt in range(HRT):
                        nc.vector.tensor_add(out=xbo[:, rt, :], in0=x_sb[:, rt, :], in1=bo_sb)

                    xmid = chunkp.tile([128, HRT, C], F32, tag="xmid", name="xmid_f")
                    for rt in range(HRT):
                        for cb in range(2):
                            po = psf.tile([128, 512], F32, tag="mm")
                            for c2 in range(4):
                                nc.tensor.matmul(
                                    po,
                                    lhsT=ctx_f[:, 2 * c2:2 * c2 + 2, rt * 128:(rt + 1) * 128],
                                    rhs=wo_sb[:, 2 * c2:2 * c2 + 2, cb * 512:(cb + 1) * 512],
                                    start=(c2 == 0), stop=(c2 == 3),
                                    perf_mode=DR,
                                )
                            sl = slice(cb * 512, (cb + 1) * 512)
                            nc.vector.tensor_scalar(out=po, in0=po, scalar1=1.0 / WS,
                                                    scalar2=None, op0=mybir.AluOpType.mult)
                            nc.vector.tensor_add(out=xmid[:, rt, sl], in0=po, in1=xbo[:, rt, sl])

                    ln2T = chunkp.tile([128, 8, HROWS], BF16, tag="ln2T", name="ln2T_f")
                    for rt in range(HRT):
                        rstd, nmr = _ln_stats(nc, smallf, xmid[:, rt, :], eps_sb)
                        lnx = smallf.tile([128, C], BF16, tag="lnx")
                        nc.scalar.activation(out=lnx, in_=xmid[:, rt, :], func=AF.Identity,
                                             bias=nmr, scale=rstd)
                        tpf = psf.tile([128, 8, 128], BF16, tag="mm")
                        for ct in range(8):
                            nc.tensor.transpose(tpf[:, ct, :], lnx[:, ct * 128:(ct + 1) * 128],
                                                ident_sb)
                        nc.vector.tensor_copy(out=ln2T[:, :, rt * 128:(rt + 1) * 128], in_=tpf)
                    return ln2T, xmid

                def emit_back(chunk, ln2T, xmid):
                    """fc + gelu + proj + residual + out DMA."""
                    r0 = chunk * HRT
                    hT = chunk1.tile([128, 32, HROWS], BF16, tag="hT", name="hT_f")
                    for htg in range(8):   # 4 fc-tiles per weight DMA
                        wf_sb = wstream.tile([128, 8, 512], BF16, tag="wfc", name="wf_sb")
                        nc.sync.dma_start(out=wf_sb, in_=wfv[:, :, htg * 512:(htg + 1) * 512])
                        for hi in range(4):
                            ht = htg * 4 + hi
                            pfc = psf2.tile([128, HROWS], F32, tag="fc")
                            for ct in range(8):
                                nc.tensor.matmul(
                                    pfc,
                                    lhsT=wf_sb[:, ct, hi * 128:(hi + 1) * 128],
                                    rhs=ln2T[:, ct, :],
                                    start=(ct == 0), stop=(ct == 7),
                                )
                            nc.scalar.activation(out=hT[:, ht, :], in_=pfc,
                                                 func=AF.Identity if sim_act else AF.Gelu,
                                                 bias=bfc_sb[:, ht:ht + 1], scale=1.0)

                    o_sb = chunk1.tile([128, HRT, C], F32, tag="o", name="o_f")
                    for rt in range(HRT):
                        for cb in range(2):
                            pp = psf.tile([128, 512], F32, tag="mm")
                            for ht in range(32):
                                nc.tensor.matmul(
                                    pp,
                                    lhsT=hT[:, ht, rt * 128:(rt + 1) * 128],
                                    rhs=wp_sb[:, ht, cb * 512:(cb + 1) * 512],
                                    start=(ht == 0), stop=(ht == 31),
                                )
                            sl = slice(cb * 512, (cb + 1) * 512)
                            nc.vector.tensor_add(out=pp, in0=pp, in1=bproj_sb[:, sl])
                            nc.vector.tensor_add(out=o_sb[:, rt, sl], in0=pp, in1=xmid[:, rt, sl])
                        nc.sync.dma_start(out=ov[:, r0 + rt, :], in_=o_sb[:, rt, :])

                if not run_F:
                    f0 = None
                else:
                    f0 = (0, *emit_front(0))
                if run_F:
                    f1 = (1, *emit_front(1))
                # wproj load issued after the first ctx DMAs so it doesn't
                # gate phase F's start; it completes under fc(chunk 0).
                if run_F:
                    nc.sync.dma_start(out=wp_sb, in_=wpv[:, :, :])
                    emit_back(*f0)
                    f2 = (2, *emit_front(2))
                    emit_back(*f1)
                    f3 = (3, *emit_front(3))
                    emit_back(*f2)
                    emit_back(*f3)
            loop_stack.close()
    nc.compile()
    return nc


class Runner:
    """Cached-jit SPMD runner (replaces run_bass_kernel_spmd for repeat calls).

    Mimics bass2jax.run_bass_via_pjrt but jits once and keeps constant inputs
    device-resident across calls. Pins a fresh neuron compile-cache dir so a
    rebuilt bass program with the same I/O signature can't hit a stale NEFF.
    """

    def __init__(self, nc, n_cores=NC):
        os.environ["NEURON_COMPILE_CACHE_URL"] = tempfile.mkdtemp(prefix="ncc_runner_")
        bass2jax.install_neuronx_cc_hook()
        self.nc = nc
        self.n_cores = n_cores
        in_names, out_names, out_avals, zero_info = [], [], [], []
        partition_name = nc.partition_id_tensor.name if nc.partition_id_tensor else None
        for alloc in nc.m.functions[0].allocations:
            if not isinstance(alloc, mybir.MemoryLocationSet):
                continue
            name = alloc.memorylocations[0].name
            if alloc.kind == "ExternalInput":
                if name != partition_name:
                    in_names.append(name)
            elif alloc.kind == "ExternalOutput":
                out_names.append(name)
                shape = tuple(alloc.tensor_shape)
                dtype = mybir.dt.np(alloc.dtype)
                out_avals.append(jax.core.ShapedArray(shape, dtype))
                zero_info.append((shape, dtype))
        self.in_names = list(in_names)
        self.out_names = out_names
        n_params = len(in_names)
        n_outs = len(out_names)
        all_in_names = in_names + out_names
        if partition_name is not None:
            all_in_names.append(partition_name)

        devices = jax.devices()[:n_cores]
        self.mesh = Mesh(np.asarray(devices), ("core",))
        self.sharding = NamedSharding(self.mesh, P("core"))

        def _body(*args):
            operands = list(args)
            if partition_name is not None:
                operands.append(bass2jax.partition_id_tensor())
            outs = bass2jax._bass_exec_p.bind(
                *operands,
                out_avals=tuple(out_avals),
                in_names=tuple(all_in_names),
                out_names=tuple(out_names),
                lowering_input_output_aliases=(),
                sim_require_finite=False,
                sim_require_nnan=False,
                nc=nc,
            )
            return tuple(outs)

        in_specs = (P("core"),) * (n_params + n_outs)
        out_specs = (P("core"),) * n_outs
        donate = tuple(range(n_params, n_params + n_outs))
        self.fn = jax.jit(
            shard_map(_body, mesh=self.mesh, in_specs=in_specs,
                      out_specs=out_specs, check_rep=False),
            donate_argnums=donate, keep_unused=True,
        )
        shardings = tuple(self.sharding for _ in zero_info)
        self.zeros_fn = jax.jit(
            lambda: tuple(jnp.zeros((n_cores * s[0], *s[1:]), d) for s, d in zero_info),
            out_shardings=shardings if zero_info else None,
        )
        self._dev_cache = {}

    def put(self, name, per_core_arrays):
        """Device-put a (replicated-or-not) input once; cached by name."""
        glob = np.concatenate([np.asarray(a) for a in per_core_arrays], axis=0)
        self._dev_cache[name] = jax.device_put(glob, self.sharding)

    def __call__(self, var_inputs):
        """var_inputs: dict name -> list of per-core np arrays (for inputs not
        previously .put()). Returns list of per-core dicts of np outputs."""
        args = []
        for name in self.in_names:
            if name in var_inputs:
                glob = np.concatenate([np.asarray(a) for a in var_inputs[name]], axis=0)
                args.append(glob)
            else:
                args.append(self._dev_cache[name])
        zeros = self.zeros_fn()
        outs = self.fn(*args, *zeros)
        res = []
        for c in range(self.n_cores):
            d = {}
            for i, name in enumerate(self.out_names):
                arr = np.asarray(outs[i])
                per = arr.shape[0] // self.n_cores
                d[name] = arr[c * per:(c + 1) * per]
            res.append(d)
        return res


_CACHE = {}


def _consts():
    if "consts" not in _CACHE:
        ident = np.eye(128, dtype=BF16NP)
        kk = np.arange(128)[:, None]
        qq = np.arange(512)[None, :]
        # per diagonal delta d: [128, 1024] = the same [128, 512] mask
        # duplicated for the two heads packed side by side in the exp tile
        masks = np.concatenate(
            [np.tile((qq >= kk + d).astype(BF16NP), (1, 2)) for d in (0, 128, 256, 384)],
            axis=1)
        _CACHE["consts"] = (ident, masks)
    return _CACHE["consts"]


def prep_weights(ln1_w, ln1_b, w_qkv, b_qkv, w_o, b_o, ln2_w, ln2_b,
                 w_fc, b_fc, w_proj, b_proj):
    """Fold LN affines into w_qkv/w_fc, pre-scale fp8 weights by WS,
    return dict name -> list of per-core arrays."""
    ident, masks = _consts()
    w_qkv = np.asarray(w_qkv, np.float32)
    w_fc = np.asarray(w_fc, np.float32)
    wqkv_eff = w_qkv * np.asarray(ln1_w, np.float32)[:, None]
    bqkv_eff = np.asarray(b_qkv, np.float32) + np.asarray(ln1_b, np.float32) @ w_qkv
    wfc_eff = w_fc * np.asarray(ln2_w, np.float32)[:, None]
    bfc_eff = np.asarray(b_fc, np.float32) + np.asarray(ln2_b, np.float32) @ w_fc

    wq_cores, bq_cores = [], []
    for c in range(NC):
        hh = c % 2
        sl = slice(hh * 512, (hh + 1) * 512)
        wq_cores.append(np.ascontiguousarray(
            (np.concatenate([wqkv_eff[:, 0 * C:1 * C][:, sl],
                             wqkv_eff[:, 1 * C:2 * C][:, sl],
                             wqkv_eff[:, 2 * C:3 * C][:, sl]], axis=1) * WS
             ).astype(E4NP)))
        bq = np.zeros((128, 12), np.float32)
        for s in range(3):
            for g in range(4):
                bq[:, s * 4 + g] = bqkv_eff[s * C + hh * 512 + g * 128:
                                            s * C + hh * 512 + (g + 1) * 128] * WS
        bq_cores.append(bq)

    wfc_arr = np.ascontiguousarray(wfc_eff.astype(BF16NP).reshape(8, 128, 4 * C)
                                   .transpose(1, 0, 2).reshape(128, 8 * 4 * C))
    return {
        "wqkv": wq_cores,
        "bqkv": bq_cores,
        "masks": [masks] * NC,
        "ident": [ident] * NC,
        "wo": [np.ascontiguousarray((np.asarray(w_o, np.float32) * WS).astype(E4NP))] * NC,
        "bo": [np.asarray(b_o, np.float32).reshape(1, C)] * NC,
        "wfc": [wfc_arr] * NC,
        "bfc": [np.ascontiguousarray(bfc_eff.reshape(32, 128).T)] * NC,
        "wproj": [np.asarray(w_proj, np.float32).astype(BF16NP)] * NC,
        "bproj": [np.asarray(b_proj, np.float32).reshape(1, C)] * NC,
    }


def prep_x(x2):
    return {
        "x": [x2[(c // 2) * T:(c // 2 + 1) * T] for c in range(NC)],
        "xr": [x2[c * RS:(c + 1) * RS] for c in range(NC)],
    }


def kernel(x, ln1_w, ln1_b, w_qkv, b_qkv, w_o, b_o,
           ln2_w, ln2_b, w_fc, b_fc, w_proj, b_proj):
    x = np.asarray(x, np.float32)
    x2 = np.ascontiguousarray(x.reshape(R, C))

    h = hashlib.blake2b(digest_size=8)
    for a in (ln1_w, ln1_b, w_qkv, b_qkv, w_o, b_o, ln2_w, ln2_b, w_fc, b_fc,
              w_proj, b_proj):
        h.update(np.ascontiguousarray(np.asarray(a, np.float32)).data)
    wkey = h.hexdigest()

    if _CACHE.get("runner") is None:
        nc = build_block()
        _CACHE["runner"] = Runner(nc)
    runner = _CACHE["runner"]

    if _CACHE.get("wkey") != wkey:
        feed = prep_weights(ln1_w, ln1_b, w_qkv, b_qkv, w_o, b_o,
                            ln2_w, ln2_b, w_fc, b_fc, w_proj, b_proj)
        for name, arrs in feed.items():
            runner.put(name, arrs)
        _CACHE["wkey"] = wkey

    xkey = hashlib.blake2b(x2.data, digest_size=8).hexdigest()
    if _CACHE.get("xkey") != xkey:
        for name, arrs in prep_x(x2).items():
            runner.put(name, arrs)
        _CACHE["xkey"] = xkey

    res = runner({})
    out = np.concatenate([res[c]["out"] for c in range(NC)], axis=0)
    return out.reshape(B, T, C)
